# revision 1
# baseline (speedup 1.0000x reference)
"""Trainium2 Bass kernel for nn_CLConv (gnn_message_passing).

Contract: kernel(**inputs) takes FULL unsharded inputs, returns the FULL
(4096, 32*max_view) float32 output.  Row-shards across 8 NeuronCores (each
core owns 512 destination rows + its slice of geodesic).

Layout: edge slot (p, j) in a [128, 64] tile holds sorted-edge l = 128*j + p,
so row(l) = 8j + p//16 and neighbour slot k = p%16.  Consequences:
 - dma_gather's native slotting (i -> out[i%128, i//128]) lands both the
   gathered x rows and geodesic blocks directly in EP layout;
 - a row's 16 edges live on 16 adjacent partitions at one j, so the weighted
   x-reduction is a per-j matmul with a block-diagonal w lhsT (PE, cheap);
 - duplicate-edge combining = PE shift-matmuls + masked adds (partition APs
   must be 32-aligned, so no direct partition shifts);
 - the transposed L2 matmul (h4-block lhsT, w2 rhs) emits vals[p, b] = edge
   128b + p, i.e. straight into EP with no repartition bounce.

Gathers use a raw InstDMAGatherAnt with elem 64B from 256B-strided tables
(7ns/idx vs 22.75 at 256B elems); geodesic is gathered as 32-fp16 blocks and
the value extracted with a one-hot mask + max-reduce.  The post-MLP tail
(L2 -> vals -> dup-combine -> weights -> reduce matmuls -> stores) is
pipelined in j-halves so the first half overlaps the second half's BN pass.
The coor tensor additionally ships in a component-major macro-block layout
(partition 32b+c, blocks 2/3 sharing base 64) so the L1 rhs is consumed
directly with no transpose bounce; its tanh runs in 4 column pieces with the
L1 psum groups regrouped by column-piece (cw=512g across all 4 blocks) so the
first matmul group starts after the first piece; exp is gated behind the last
BN quarter so it never stalls the in-order Act stream mid-MLP.
Verified on TRN2 hardware: rel err 1.26e-3; TimelineSim 28125 ns/core
(baseline 82124).
"""

import os
import sys

sys.path.insert(0, "/opt/trn_rl_repo")
# ASAP tile scheduler places cross-engine waits far better for this kernel
# (the legacy CoreSim-flow scheduler stalls the Act stream on gather sems).
os.environ.setdefault("TILE_SCHEDULER", "asap")

import numpy as np

import concourse.bacc as bacc
import concourse.bass as bass
import concourse.mybir as mybir
import concourse.tile as tile

N = 4096
KN = 16
F = 32
H = 64
NCORES = 8
NLOC = N // NCORES          # 512 rows/core
ELOC = NLOC * KN            # 8192 edges/core
P = 128
JW = ELOC // P              # 64 j-slots
BN_EPS = 1e-5

f32 = mybir.dt.float32
fp16 = mybir.dt.float16
i16 = mybir.dt.int16
AB = mybir.AluOpType
AF = mybir.ActivationFunctionType


def _raw_dma_gather(eng, out_ap, in_ap, idxs_ap, num_idxs, elem_size, elem_step,
                    queue_num=0):
    """dma_gather without the elem_size_bytes%256 assert (table row stride must
    still be a multiple of 256B -- that is a real descriptor field)."""
    assert idxs_ap.dtype == mybir.dt.int16
    assert in_ap.dtype == out_ap.dtype
    assert in_ap.ap[-1][1] == elem_size
    assert in_ap.ap[0][0] == elem_step
    stride_bytes = elem_step * mybir.dt.size(in_ap.dtype)
    assert stride_bytes % 256 == 0
    _in_ap = eng.lower_ap_dma(in_ap, for_custom_bir_dma=True)
    _idxs_ap = eng.lower_ap(idxs_ap)
    _out_ap = eng.lower_ap(out_ap)
    return eng.add_instruction(
        mybir.InstDMAGatherAnt(
            name=eng.bass.get_next_instruction_name(),
            ins=[*_in_ap, _idxs_ap, eng.lower_val_access(eng.to_reg(num_idxs))],
            outs=[_out_ap],
            transpose=False, num_idxs=num_idxs, elem_size=elem_size,
            stride_bytes_256=stride_bytes // 256,
            gen_mode=0, single_packet=False, queue_num=queue_num))


def _layout(dmax):
    """fp16 column offsets in the packed per-partition constant tensors."""
    a = {}
    o = 0
    for name, w in (("ce", 4 * JW), ("qrep", 4 * JW), ("ident", P),
                    ("w01", H), ("w01x", H), ("qmask", 8), ("w2", 1),
                    ("pad", 1), ("bnsc", 4), ("b01", 2)):
        a[name] = o
        o += w
    a["_W"] = (o + 15) // 16 * 16
    g = {}
    o = 0
    for name, w in (("gx", ELOC // 16), ("gg0", ELOC // 32),
                    ("gg1", ELOC // 32)):
        g[name] = o
        o += w
    g["_W"] = (o + 15) // 16 * 16
    b = {}
    o = 0
    for name, w in (("mask", 32 * JW), ("shf", 2 * dmax * P),
                    ("dupm", 2 * dmax * JW), ("rep", JW)):
        b[name] = o
        o += w
    b["_W"] = (o + 15) // 16 * 16
    return a, g, b


def _build_program(dmax, n_views, attn_consts, sim=False):
    nc = bacc.Bacc("TRN2", target_bir_lowering=False, debug=False,
                   num_swdge_queues=2)
    A, G, B = _layout(dmax)
    nshift = 2 * dmax

    ce5t = nc.dram_tensor("ce5t", [P, 2048], i16, kind="ExternalInput")
    cst0 = nc.dram_tensor("cst0", [P, A["_W"]], i16, kind="ExternalInput")
    cstg = nc.dram_tensor("cstg", [P, G["_W"]], i16, kind="ExternalInput")
    cstb = nc.dram_tensor("cstb", [P, B["_W"]], i16, kind="ExternalInput")
    xtab = nc.dram_tensor("xtab", [N, 128], fp16, kind="ExternalInput")
    gtab0 = nc.dram_tensor("gtab0", [NLOC * N // 64, 128], fp16, kind="ExternalInput")
    gtab1 = nc.dram_tensor("gtab1", [NLOC * N // 64, 128], fp16, kind="ExternalInput")
    # feature-major fp16; host transposes/casts to (NLOC, n_views*F) f32
    out = nc.dram_tensor("out", [n_views * F, NLOC], fp16, kind="ExternalOutput")

    a00, a01, a10, a11, u0, u1, v0, v1, w0s, b2f = [float(z) for z in attn_consts]

    with tile.TileContext(nc) as tc:
        with (tc.tile_pool(name="sbC", bufs=1) as sbc,
              tc.tile_pool(name="sbG", bufs=1) as sbg,
              tc.tile_pool(name="sbM", bufs=1) as sbm,
              tc.tile_pool(name="sbE", bufs=1) as sbe,
              tc.tile_pool(name="sbO", bufs=1) as sbo,
              tc.tile_pool(name="ps1", bufs=2, space="PSUM") as ps1,
              tc.tile_pool(name="psh", bufs=1, space="PSUM") as psh,
              tc.tile_pool(name="psr", bufs=2, space="PSUM") as psr):
            # ---- constant loads; coor ships twice: EP layout (attention)
            # and component-major macro-block layout ce5 (partition 32b+c,
            # 2048 edge-cols) so the L1 rhs needs NO transpose bounce ----
            c5 = sbm.tile([P, 2048], i16)
            nc.sync.dma_start(out=c5[:], in_=ce5t[:])
            ca = sbc.tile([P, A["_W"]], i16)
            nc.sync.dma_start(out=ca[:], in_=cst0[:])
            cg = sbc.tile([P, G["_W"]], i16)
            nc.sync.dma_start(out=cg[:], in_=cstg[:])

            def caf(name, w):
                return ca[:, A[name]:A[name] + w].bitcast(fp16)

            th5 = sbm.tile([P, 2048], fp16)
            for q in range(4):
                nc.scalar.activation(out=th5[:, 512 * q:512 * (q + 1)],
                                     in_=c5[:, 512 * q:512 * (q + 1)].bitcast(fp16),
                                     func=AF.Tanh)

            cb = sbc.tile([P, B["_W"]], i16)
            nc.sync.dma_start(out=cb[:], in_=cstb[:])

            # ---- gathers (SWDGE; all EP-slotted) ----
            gblk = sbg.tile([P, JW * 32], fp16)   # [p, j*32+v]
            for h, gt in ((0, gtab0), (1, gtab1)):
                _raw_dma_gather(
                    nc.gpsimd,
                    gblk[:, 1024 * h:1024 * (h + 1)].rearrange(
                        "p (j v) -> p j v", v=32),
                    gt[:, 0:32], cg[:, G["gg" + str(h)]:G["gg" + str(h)] + ELOC // 32],
                    ELOC // 2, 32, 128, queue_num=1)
            xg = sbg.tile([P, JW * F], fp16)      # [p, j*32+f]
            _raw_dma_gather(
                nc.gpsimd, xg[:].rearrange("p (j f) -> p j f", f=F),
                xtab[:, 0:F], cg[:, G["gx"]:G["gx"] + ELOC // 16],
                ELOC, F, 128, queue_num=0)


            def cbf(name, w):
                return cb[:, B[name]:B[name] + w].bitcast(fp16)

            w01 = ca[0:4, A["w01"]:A["w01"] + H].bitcast(fp16)
            b01 = ca[:, A["b01"]:A["b01"] + 2].bitcast(f32)
            bnsc = ca[:, A["bnsc"]:A["bnsc"] + 4].bitcast(f32)
            w2c = caf("w2", 1)
            qmask = caf("qmask", 8)

            h4 = sbm.tile([P, ELOC // 2], fp16)
            VA = sbe.tile([P, 2 * JW], fp16)   # [vals | alpha]
            VS = sbe.tile([P, 2 * JW], fp16)

            # ---- attention -> alpha in VA[:, JW:2JW] (DVE, early) ----
            SQ = sbe.tile([P, 2 * JW], fp16)
            nc.vector.tensor_tensor(out=SQ[:], in0=caf("qrep", 2 * JW),
                                    in1=ca[:, A["qrep"] + 2 * JW:A["qrep"] + 4 * JW].bitcast(fp16),
                                    op=AB.add)
            SXY = sbe.tile([P, 2 * JW], fp16)
            nc.vector.tensor_tensor(out=SXY[:], in0=caf("ce", 2 * JW),
                                    in1=ca[:, A["ce"] + 2 * JW:A["ce"] + 4 * JW].bitcast(fp16),
                                    op=AB.add)
            qa = sbe.tile([P, JW], fp16)
            qb = sbe.tile([P, JW], fp16)
            qc = sbe.tile([P, JW], fp16)
            for qt, cx, cy, cc in ((qa, a00, a10, v0), (qb, a01, a11, v1),
                                   (qc, u0, u1, w0s)):
                nc.vector.tensor_scalar(out=qt[:], in0=SQ[:, 0:JW], scalar1=cx,
                                        scalar2=cc, op0=AB.mult, op1=AB.add)
                nc.vector.scalar_tensor_tensor(out=qt[:], in0=SQ[:, JW:2 * JW],
                                               scalar=cy, in1=qt[:],
                                               op0=AB.mult, op1=AB.add)
            m1 = sbe.tile([P, JW], fp16)
            nc.vector.tensor_tensor(out=m1[:], in0=SXY[:, 0:JW], in1=qa[:],
                                    op=AB.mult)
            m2 = sbe.tile([P, JW], fp16)
            nc.vector.tensor_tensor(out=m2[:], in0=SXY[:, JW:2 * JW], in1=qb[:],
                                    op=AB.mult)
            nc.vector.tensor_tensor(out=m1[:], in0=m1[:], in1=m2[:], op=AB.add)
            nc.vector.tensor_tensor(out=m1[:], in0=m1[:], in1=qc[:], op=AB.add)
            nc.vector.tensor_scalar(out=m2[:], in0=m1[:], scalar1=-1.0,
                                    scalar2=None, op0=AB.mult)
            nc.vector.tensor_tensor(out=VA[:, JW:2 * JW], in0=m1[:], in1=m2[:],
                                    op=AB.max)

            dupm = cbf("dupm", nshift * JW)

            def dup_combine(col0, dtag):
                """VS[:, col0:col0+JW] = dup-group sums of VA[:, col0:col0+JW]."""
                if dmax == 0:
                    nc.vector.tensor_copy(out=VS[:, col0:col0 + JW],
                                          in_=VA[:, col0:col0 + JW])
                    return
                dsum = sbe.tile([P, nshift * JW], fp16, name=f"ds{dtag}",
                                tag=f"ds{dtag}")
                shp = psh.tile([P, nshift * JW], f32, tag="shpa", name=f"shp{dtag}")
                for s in range(nshift):
                    S = cb[:, B["shf"] + s * P:
                           B["shf"] + (s + 1) * P].bitcast(fp16)
                    nc.tensor.matmul(out=shp[:, JW * s:JW * (s + 1)],
                                     lhsT=S, rhs=VA[:, col0:col0 + JW],
                                     start=True, stop=True)
                nc.vector.tensor_tensor(out=dsum[:], in0=shp[:],
                                        in1=dupm[:], op=AB.mult)
                w = nshift * JW
                while w > JW:
                    nc.vector.tensor_tensor(out=dsum[:, 0:w // 2],
                                            in0=dsum[:, 0:w // 2],
                                            in1=dsum[:, w // 2:w], op=AB.add)
                    w //= 2
                nc.vector.tensor_tensor(out=VS[:, col0:col0 + JW],
                                        in0=VA[:, col0:col0 + JW],
                                        in1=dsum[:, 0:JW], op=AB.add)


            # ---- geodesic extraction (early; per gather half) ----
            geo = sbe.tile([P, JW], fp16)
            for h in range(2):
                gm = sbe.tile([P, 16 * JW], fp16, name=f"gm{h}", tag=f"gm{h}")
                nc.vector.tensor_tensor(
                    out=gm[:], in0=gblk[:, 1024 * h:1024 * (h + 1)],
                    in1=cbf("mask", 32 * JW)[:, 1024 * h:1024 * (h + 1)],
                    op=AB.mult)
                with nc.allow_low_precision(reason="one-hot max-select"):
                    nc.vector.tensor_reduce(
                        out=geo[:, 32 * h:32 * (h + 1)].rearrange(
                            "p (j o) -> p j o", o=1),
                        in_=gm[:].rearrange("p (j v) -> p j v", v=32),
                        axis=mybir.AxisListType.X, op=AB.max)



            # ---- L1 (5->64 folded) + tanh + BN*tanh, feature-major ----
            h2 = sbm.tile([P, ELOC // 2], fp16)
            for g in range(4):
                pt = ps1.tile([P, 1024], f32, tag="l1")
                for b in range(4):
                    t, u, cw = b % 2, b // 2, 512 * g
                    if b < 2:
                        base, kk, wf = 32 * b, 4, "w01"
                    else:
                        base, kk, wf = 64, 8, ("w01" if b == 2 else "w01x")
                    w01b = ca[base:base + kk,
                              A[wf]:A[wf] + H].bitcast(fp16)
                    nc.tensor.matmul(
                        out=pt[64 * t:64 * t + 64, 512 * u:512 * (u + 1)],
                        lhsT=w01b,
                        rhs=th5[base:base + kk, cw:cw + 512],
                        start=True, stop=True)
                nc.scalar.activation(out=h2[:, 1024 * g:1024 * (g + 1)],
                                     in_=pt[:], func=AF.Tanh,
                                     bias=b01[:, 0:1])
                nc.scalar.activation(out=h4[:, 1024 * g:1024 * (g + 1)],
                                     in_=h2[:, 1024 * g:1024 * (g + 1)],
                                     func=AF.Tanh, scale=bnsc[:, 0:1],
                                     bias=bnsc[:, 1:2])

            # alpha-half dup combining + decay argument (PE slots after L1)
            dup_combine(JW, "a")
            wdec = sbe.tile([P, JW], fp16)
            nc.vector.tensor_tensor(out=wdec[:], in0=VS[:, JW:2 * JW],
                                    in1=geo[:], op=AB.mult)

            # gate exp on the last bn quarter (0*h4) so the in-order Act
            # stream never stalls mid-MLP waiting for the geodesic chain
            wdecx = sbe.tile([P, JW], fp16)
            nc.vector.scalar_tensor_tensor(
                out=wdecx[:], in0=h4[:, ELOC // 2 - JW:ELOC // 2], scalar=0.0,
                in1=wdec[:], op0=AB.mult, op1=AB.add)
            dec = sbe.tile([P, JW], fp16)
            nc.scalar.activation(out=dec[:], in_=wdecx[:], func=AF.Exp,
                                 scale=-1.0)
            nc.gpsimd.tensor_tensor(out=dec[:], in0=dec[:],
                                     in1=cbf("rep", JW), op=AB.mult)

            # ---- L2 transposed + per-half tail pipeline ----
            vps = ps1.tile([P, 1024], f32, tag="l1", name="vps")
            outv = out[:].rearrange("(v f) r -> v f r", v=n_views)
            wv = [sbe.tile([P, JW], fp16, name=f"w{v+1}", tag=f"w{v+1}")
                  for v in range(n_views)]
            wds = [sbe.tile([P, 8 * JW], fp16, name=f"wd{v}", tag=f"wd{v}")
                   for v in range(n_views)]
            HW2 = JW // 2
            for hh in range(2):
                j0 = HW2 * hh
                js = slice(j0, j0 + HW2)
                for j in range(j0, j0 + HW2):
                    t = (j // 16) % 2
                    cols = 1024 * ((j // 4) % 4) + 512 * (j // 32) + 128 * (j % 4)
                    nc.tensor.matmul(out=vps[:, j:j + 1],
                                     lhsT=h4[64 * t:64 * t + 64, cols:cols + 128],
                                     rhs=w2c[64 * t:64 * t + 64, :],
                                     start=True, stop=True)
                nc.vector.tensor_scalar(out=VA[:, js], in0=vps[:, js],
                                        scalar1=b2f, scalar2=0.0,
                                        op0=AB.add, op1=AB.max)
                if dmax > 0:
                    dsum = sbe.tile([P, nshift * HW2], fp16, name=f"dsv{hh}",
                                    tag=f"dsv{hh}")
                    shp = psh.tile([P, nshift * HW2], f32, tag="shp",
                                   name=f"shpv{hh}")
                    for s in range(nshift):
                        S = cb[:, B["shf"] + s * P:
                               B["shf"] + (s + 1) * P].bitcast(fp16)
                        nc.tensor.matmul(out=shp[:, HW2 * s:HW2 * (s + 1)],
                                         lhsT=S, rhs=VA[:, js],
                                         start=True, stop=True)
                    nc.vector.tensor_tensor(
                        out=dsum[:].rearrange("p (s j) -> p s j", j=HW2),
                        in0=shp[:].rearrange("p (s j) -> p s j", j=HW2),
                        in1=dupm[:].rearrange("p (s j) -> p s j", j=JW)[:, :, js],
                        op=AB.mult)
                    fold = sbe.tile([P, HW2], fp16, name=f"fdv{hh}",
                                    tag=f"fdv{hh}")
                    with nc.allow_low_precision(reason="<=4-term dup sums"):
                        nc.vector.tensor_reduce(
                            out=fold[:].rearrange("p (j o) -> p j o", o=1),
                            in_=dsum[:].rearrange("p (s j) -> p j s", j=HW2),
                            axis=mybir.AxisListType.X, op=AB.add)
                    nc.vector.tensor_tensor(out=VS[:, js], in0=VA[:, js],
                                            in1=fold[:], op=AB.add)
                else:
                    nc.vector.tensor_copy(out=VS[:, js], in_=VA[:, js])
                weng = nc.gpsimd if hh == 0 else nc.vector
                weng.tensor_tensor(out=wv[0][:, js], in0=VS[:, js],
                                   in1=dec[:, js], op=AB.mult)
                for v in range(1, n_views):
                    weng.tensor_tensor(out=wv[v][:, js],
                                       in0=wv[v - 1][:, js],
                                       in1=wv[0][:, js], op=AB.mult)
                for v in range(n_views):
                    nc.vector.tensor_tensor(
                        out=wds[v][:].rearrange("p (j q) -> p j q", q=8)[:, js],
                        in0=wv[v][:, js].rearrange("p (j o) -> p j o", o=1)
                            .to_broadcast([P, HW2, 8]),
                        in1=qmask[:, None, :].to_broadcast([P, HW2, 8]),
                        op=AB.mult)
                rsl = slice(8 * j0, 8 * (j0 + HW2))
                for v in range(n_views):
                    rps = psr.tile([F, NLOC // 2], f32, tag="rps",
                                   name=f"rps{v}h{hh}")
                    for jj in range(HW2):
                        j = j0 + jj
                        nc.tensor.matmul(out=rps[:, 8 * jj:8 * (jj + 1)],
                                         lhsT=xg[:, F * j:F * (j + 1)],
                                         rhs=wds[v][:, 8 * j:8 * (j + 1)],
                                         start=True, stop=True)
                    ov = sbo.tile([F, NLOC // 2], fp16, name=f"ov{v}h{hh}",
                                  tag=f"ov{v}h{hh}")
                    if v % 2 == 0 and not (hh == 1 and v == n_views - 1):
                        nc.scalar.copy(out=ov[:], in_=rps[:])
                    else:
                        nc.vector.tensor_copy(out=ov[:], in_=rps[:])
                    eng = nc.sync if (v + hh) % 2 == 0 else nc.scalar
                    eng.dma_start(out=outv[v][:, rsl], in_=ov[:])

    nc.compile()
    return nc


def _prepare(inputs):
    """Host-side staging: edge sort, gather tables/indices, masks, weight folds."""
    x = np.asarray(inputs["x"], np.float32)
    coor = np.asarray(inputs["local_graph_coor"], np.float32)
    sparse_idx = np.asarray(inputs["sparse_idx"])
    geodesic = np.asarray(inputs["geodesic"], np.float32)
    angle_ratio = float(np.asarray(inputs["angle_ratio"]).ravel()[0])
    Wq = np.asarray(inputs["Wq"], np.float32); bq = np.asarray(inputs["bq"], np.float32)
    Wk = np.asarray(inputs["Wk"], np.float32); bk = np.asarray(inputs["bk"], np.float32)
    W0 = np.asarray(inputs["W0"], np.float32); b0 = np.asarray(inputs["b0"], np.float32)
    W1 = np.asarray(inputs["W1"], np.float32); b1 = np.asarray(inputs["b1"], np.float32)
    bn_g = np.asarray(inputs["bn_g"], np.float32); bn_b = np.asarray(inputs["bn_b"], np.float32)
    bn_m = np.asarray(inputs["bn_m"], np.float32); bn_v = np.asarray(inputs["bn_v"], np.float32)
    W2 = np.asarray(inputs["W2"], np.float32); b2 = np.asarray(inputs["b2"], np.float32)
    n_views = int(np.asarray(inputs["max_view"]).ravel()[0])

    col = np.asarray(sparse_idx[1], np.int64).reshape(N, KN)
    order = np.argsort(col, axis=1, kind="stable")
    col_s = np.take_along_axis(col, order, axis=1)                  # (N, K)
    eidx = (np.arange(N)[:, None] * KN + order).reshape(-1)
    coor_s = coor[eidx]                                             # (E, 4)

    same_prev = np.zeros((N, KN), bool)
    same_prev[:, 1:] = col_s[:, 1:] == col_s[:, :-1]
    rep = (~same_prev).astype(np.float32) * angle_ratio
    run = np.zeros((N, KN), np.int64)
    for k in range(1, KN):
        run[:, k] = np.where(same_prev[:, k], run[:, k - 1] + 1, 0)
    dmax = int(run.max())

    Aq = Wq @ Wk.T
    u = Wq @ bk
    vv = Wk @ bq
    attn_consts = (Aq[0, 0], Aq[0, 1], Aq[1, 0], Aq[1, 1], u[0], u[1],
                   vv[0], vv[1], float(bq @ bk), float(b2.ravel()[0]))

    bns = (bn_g / np.sqrt(bn_v + BN_EPS)).astype(np.float32)
    bnc = (bn_b - bn_m * bns).astype(np.float32)
    W01 = (W0 @ W1).astype(np.float32)                              # (4, H)
    b01 = (b0 @ W1 + b1).astype(np.float32)                         # (H,)

    A, G, B = _layout(dmax)
    pidx = np.arange(P)

    def wrap16(lst):
        lst = np.asarray(lst, np.int64)
        return np.tile(lst.reshape(len(lst) // 16, 16).T, (8, 1)).astype(np.int16)

    xtab = np.zeros((N, 128), np.float16)
    xtab[:, :F] = x.astype(np.float16)

    in_maps = []
    for c in range(NCORES):
        r0 = c * NLOC
        colc = col_s[r0:r0 + NLOC].reshape(-1)                      # (ELOC,) l-order
        coorc = coor_s[r0 * KN:(r0 + NLOC) * KN]                    # (ELOC, 4)
        repc = rep[r0:r0 + NLOC].reshape(-1)
        gsh = geodesic[r0:r0 + NLOC]                                # (512, 4096)
        rows_l = np.arange(ELOC) // KN                              # local row per edge
        k_l = np.arange(ELOC) % KN

        ce5_a = np.zeros((P, 2048), np.float16)
        c5v = coorc.astype(np.float16).reshape(4, 2048, 4)   # (b, e', c)
        for b, r0_ in enumerate((0, 32, 64, 68)):
            ce5_a[r0_:r0_ + 4, :] = c5v[b].T
        cst0_a = np.zeros((P, A["_W"]), np.float16)
        ce = coorc.astype(np.float16)                               # (ELOC, 4)
        cst0_a[:, A["ce"]:A["ce"] + 4 * JW] = \
            ce.reshape(JW, P, 4).transpose(1, 2, 0).reshape(P, 4 * JW)
        qco = coor[(np.arange(r0, r0 + NLOC)) * KN].astype(np.float16)  # slot0 coor
        qrep = qco[rows_l]                                          # (ELOC, 4)
        cst0_a[:, A["qrep"]:A["qrep"] + 4 * JW] = \
            qrep.reshape(JW, P, 4).transpose(1, 2, 0).reshape(P, 4 * JW)
        cst0_a[:, A["ident"]:A["ident"] + P] = np.eye(P, dtype=np.float16)
        for r0_ in (0, 32, 64):
            cst0_a[r0_:r0_ + 4, A["w01"]:A["w01"] + H] = W01.astype(np.float16)
        cst0_a[68:72, A["w01x"]:A["w01x"] + H] = W01.astype(np.float16)
        cst0_a[:, A["qmask"]:A["qmask"] + 8] = \
            (pidx[:, None] // 16 == np.arange(8)[None, :]).astype(np.float16)
        cst0_a[:, A["w2"]:A["w2"] + 1] = \
            np.tile(W2[:, 0], 2).astype(np.float16)[:, None]
        bnsc = np.stack([np.tile(bns, 2), np.tile(bnc, 2)], 1).astype(np.float32)
        cst0_a[:, A["bnsc"]:A["bnsc"] + 4] = bnsc.view(np.float16)
        cst0_a[:, A["b01"]:A["b01"] + 2] = \
            np.tile(b01, 2).astype(np.float32)[:, None].view(np.float16)

        cstg_a = np.zeros((P, G["_W"]), np.float16)
        cstg_a[:, G["gx"]:G["gx"] + ELOC // 16] = wrap16(colc).view(np.float16)
        for h in range(2):
            el = np.arange(h * ELOC // 2, (h + 1) * ELOC // 2)
            gidx = (rows_l[el] % 256) * 128 + colc[el] // 32
            cstg_a[:, G["gg" + str(h)]:G["gg" + str(h)] + ELOC // 32] = \
                wrap16(gidx).view(np.float16)

        cstb_a = np.zeros((P, B["_W"]), np.float16)
        onehot = (colc % 32)[:, None] == np.arange(32)[None, :]     # (ELOC, 32)
        cstb_a[:, B["mask"]:B["mask"] + 32 * JW] = \
            onehot.astype(np.float16).reshape(JW, P, 32).transpose(1, 0, 2) \
            .reshape(P, 32 * JW)
        for s in range(2 * dmax):
            d = s + 1 if s < dmax else -(s - dmax + 1)
            # shift matrix: out[m, c] = VA[m+d, c]  -> S[k, m] = [k == m+d]
            S = (pidx[:, None] == pidx[None, :] + d).astype(np.float16)
            cstb_a[:, B["shf"] + s * P:B["shf"] + (s + 1) * P] = S
            # mask for dst edge l=128j+p absorbing l+d (same row & col)
            msk = np.zeros(ELOC, np.float32)
            if d > 0:
                msk[:ELOC - d] = ((k_l[:ELOC - d] + d < KN) &
                                  (colc[d:] == colc[:-d]))
            else:
                dd = -d
                msk[dd:] = ((k_l[dd:] - dd >= 0) & (colc[:-dd] == colc[dd:]))
            cstb_a[:, B["dupm"] + s * JW:B["dupm"] + (s + 1) * JW] = \
                np.ascontiguousarray(msk.reshape(JW, P).T).astype(np.float16)
        cstb_a[:, B["rep"]:B["rep"] + JW] = \
            np.ascontiguousarray(repc.reshape(JW, P).T).astype(np.float16)

        gt = []
        for h in range(2):
            g4 = gsh[256 * h:256 * (h + 1)].reshape(NLOC * N // 64, 32)
            gta = np.zeros((NLOC * N // 64, 128), np.float16)
            gta[:, :32] = g4.astype(np.float16)
            gt.append(gta)

        in_maps.append({"ce5t": ce5_a.view(np.int16),
                        "cst0": cst0_a.view(np.int16),
                        "cstg": cstg_a.view(np.int16),
                        "cstb": cstb_a.view(np.int16),
                        "xtab": xtab, "gtab0": gt[0], "gtab1": gt[1]})
    return in_maps, dmax, n_views, attn_consts


def kernel(**inputs):
    from concourse.bass_utils import run_bass_kernel_spmd
    in_maps, dmax, n_views, attn_consts = _prepare(inputs)
    nc = _build_program(dmax, n_views, attn_consts, sim=False)
    res = run_bass_kernel_spmd(nc, in_maps, list(range(NCORES)))
    return np.vstack([np.asarray(res.results[c]["out"]).T.astype(np.float32)
                      for c in range(NCORES)])



# revision 53
# speedup vs baseline: 1.2039x; 1.2039x over previous
"""Trainium2 Bass kernel for nn_CLConv (gnn_message_passing).

Contract: kernel(**inputs) takes FULL unsharded inputs, returns the FULL
(4096, 32*max_view) float32 output.  Row-shards across 8 NeuronCores (each
core owns 512 destination rows + its slice of geodesic).

Layout: edge slot (p, j) in a [128, 64] tile holds sorted-edge l = 128*j + p,
so row(l) = 8j + p//16 and neighbour slot k = p%16.  dma_gather's native
slotting (i -> out[i%128, i//128]) lands both the gathered x rows and
geodesic blocks directly in EP layout.

v2 schedule (cost-model driven):
 - index tables load FIRST so SWDGE desc-gen (the serial bottleneck: 994ns
   + 0.34ns/desc on the gpsimd engine) starts ~3us;
 - gather order geo0, geo1, then x split in halves (the last gather's
   transfer pipelines against its own second half's desc-gen);
 - tail: dec8 = dec*rep broadcast over the 8 q-slots once; wds1 = VS
   broadcast * dec8; wds2 = wds1^2, wds3 = wds1*wds2 (one-hot qmask is
   idempotent so squaring replicated weights is exact);
 - one reduce matmul per j with a 24-wide rhs (3 views side by side) into a
   [32, 1536] psum; per-view strided copies split across DVE/Act/Pool; one
   [96, 512] fp16 store;
 - PE primed with junk matmuls so L1 runs at the fast p-state.
"""

import os
import sys

sys.path.insert(0, "/opt/trn_rl_repo")
# the legacy CoreSim-flow scheduler HOL-blocks far less on the PE/DVE queues
# for this structure than the asap list scheduler
os.environ["TILE_SCHEDULER"] = ""

import numpy as np

import concourse.bacc as bacc
import concourse.bass as bass
import concourse.mybir as mybir
import concourse.tile as tile

N = 4096
KN = 16
F = 32
H = 64
NCORES = 8
NLOC = N // NCORES          # 512 rows/core
ELOC = NLOC * KN            # 8192 edges/core
P = 128
JW = ELOC // P              # 64 j-slots
BN_EPS = 1e-5

f32 = mybir.dt.float32
fp16 = mybir.dt.float16
i16 = mybir.dt.int16
AB = mybir.AluOpType
AF = mybir.ActivationFunctionType


def _raw_dma_gather(eng, out_ap, in_ap, idxs_ap, num_idxs, elem_size, elem_step,
                    queue_num=0):
    """dma_gather without the elem_size_bytes%256 assert (table row stride must
    still be a multiple of 256B -- that is a real descriptor field)."""
    assert idxs_ap.dtype == mybir.dt.int16
    assert in_ap.dtype == out_ap.dtype
    assert in_ap.ap[-1][1] == elem_size
    assert in_ap.ap[0][0] == elem_step
    stride_bytes = elem_step * mybir.dt.size(in_ap.dtype)
    assert stride_bytes % 256 == 0
    _in_ap = eng.lower_ap_dma(in_ap, for_custom_bir_dma=True)
    _idxs_ap = eng.lower_ap(idxs_ap)
    _out_ap = eng.lower_ap(out_ap)
    return eng.add_instruction(
        mybir.InstDMAGatherAnt(
            name=eng.bass.get_next_instruction_name(),
            ins=[*_in_ap, _idxs_ap, eng.lower_val_access(eng.to_reg(num_idxs))],
            outs=[_out_ap],
            transpose=False, num_idxs=num_idxs, elem_size=elem_size,
            stride_bytes_256=stride_bytes // 256,
            gen_mode=0, single_packet=False, queue_num=queue_num))


def _layout(dmax):
    """fp16 column offsets in the packed per-partition constant tensors."""
    a = {}
    o = 0
    for name, w in (("ce", 4 * JW), ("qrep", 4 * JW),
                    ("qmask", 8), ("w2", 1),
                    ("pad", 1), ("bnsc", 4), ("b01", 2)):
        a[name] = o
        o += w
    a["_W"] = (o + 15) // 16 * 16
    g = {}
    o = 0
    for name, w in (("gg0", ELOC // 32), ("gg1", ELOC // 32),
                    ("gx", ELOC // 16)):
        g[name] = o
        o += w
    g["_W"] = (o + 15) // 16 * 16
    b = {}
    o = 0
    for name, w in (("mask", 32 * JW), ("shf", 2 * dmax * P),
                    ("dupm", 2 * dmax * JW), ("rep", JW)):
        b[name] = o
        o += w
    b["_W"] = (o + 15) // 16 * 16
    return a, g, b


def _build_program(dmax, n_views, attn_consts, sim=False):
    nc = bacc.Bacc("TRN2", target_bir_lowering=False, debug=False,
                   num_swdge_queues=2)
    A, G, B = _layout(dmax)
    nshift = 2 * dmax

    cstg = nc.dram_tensor("cstg", [P, G["_W"]], i16, kind="ExternalInput")
    ce5t = nc.dram_tensor("ce5t", [P, 256], i16, kind="ExternalInput")
    w01t = nc.dram_tensor("w01t", [P, 16 * H], i16, kind="ExternalInput")
    cst0 = nc.dram_tensor("cst0", [P, A["_W"]], i16, kind="ExternalInput")
    cstb = nc.dram_tensor("cstb", [P, B["_W"]], i16, kind="ExternalInput")
    xtab = nc.dram_tensor("xtab", [N, 128], fp16, kind="ExternalInput")
    gtab0 = nc.dram_tensor("gtab0", [NLOC * N // 64, 128], fp16, kind="ExternalInput")
    gtab1 = nc.dram_tensor("gtab1", [NLOC * N // 64, 128], fp16, kind="ExternalInput")
    # [32v+f, r] fp16; host transposes/casts to (NLOC, n_views*F) f32
    out = nc.dram_tensor("out", [n_views * F, NLOC], fp16, kind="ExternalOutput")

    a00, a01, a10, a11, u0, u1, v0, v1, w0s, b2f = [float(z) for z in attn_consts]

    with tile.TileContext(nc) as tc:
        with (tc.tile_pool(name="sbC", bufs=1) as sbc,
              tc.tile_pool(name="sbG", bufs=1) as sbg,
              tc.tile_pool(name="sbM", bufs=1) as sbm,
              tc.tile_pool(name="sbE", bufs=1) as sbe,
              tc.tile_pool(name="sbO", bufs=1) as sbo,
              tc.tile_pool(name="ps1", bufs=2, space="PSUM") as ps1,
              tc.tile_pool(name="psh", bufs=1, space="PSUM") as psh,
              tc.tile_pool(name="psr", bufs=1, space="PSUM") as psr):
            # ---- loads: coor first (Act chain), geodesic gather indices
            # second (desc-gen is the serial bottleneck), then weights,
            # x indices, masks ----
            c5 = sbm.tile([P, 256], i16)
            nc.sync.dma_start(out=c5[:], in_=ce5t[:])
            cg = sbc.tile([P, G["_W"]], i16)
            nc.sync.dma_start(out=cg[:, 0:ELOC // 16],
                              in_=cstg[:, 0:ELOC // 16])
            wv = sbc.tile([P, 16 * H], i16)
            nc.sync.dma_start(out=wv[:], in_=w01t[:])
            ca = sbc.tile([P, A["_W"]], i16)
            nc.sync.dma_start(out=ca[:], in_=cst0[:])
            nc.sync.dma_start(out=cg[:, ELOC // 16:G["_W"]],
                              in_=cstg[:, ELOC // 16:G["_W"]])
            cb = sbc.tile([P, B["_W"]], i16)
            nc.sync.dma_start(out=cb[:], in_=cstb[:])

            # ---- PE priming: junk matmuls so the p-state ramps to fast
            # before the real L1 matmuls arrive ----
            scr = sbg.tile([P, 512], fp16)
            nc.vector.memset(scr[:], 0.0)
            for i in range(7):
                pp = psr.tile([P, 512], f32, tag=f"rp{i % 3}", name=f"prime{i}")
                nc.tensor.matmul(out=pp[:], lhsT=scr[:, 0:128], rhs=scr[:],
                                 start=True, stop=True)

            # ---- gathers (SWDGE on gpsimd): geo halves first, then x in
            # halves so the last transfer overlaps desc-gen ----
            gblk = sbg.tile([P, JW * 32], fp16)   # [p, j*32+v]
            for h, gt in ((0, gtab0), (1, gtab1)):
                _raw_dma_gather(
                    nc.gpsimd,
                    gblk[:, 1024 * h:1024 * (h + 1)].rearrange(
                        "p (j v) -> p j v", v=32),
                    gt[:, 0:32], cg[:, G["gg" + str(h)]:G["gg" + str(h)] + ELOC // 32],
                    ELOC // 2, 32, 128, queue_num=1)
            xg = sbg.tile([P, JW * F], fp16)      # [p, j*32+f]
            for h in range(2):
                _raw_dma_gather(
                    nc.gpsimd,
                    xg[:, 1024 * h:1024 * (h + 1)].rearrange(
                        "p (j f) -> p j f", f=F),
                    xtab[:, 0:F],
                    cg[:, G["gx"] + h * ELOC // 32:G["gx"] + (h + 1) * ELOC // 32],
                    ELOC // 2, F, 128, queue_num=0)

            def caf(name, w):
                return ca[:, A[name]:A[name] + w].bitcast(fp16)

            def cbf(name, w):
                return cb[:, B[name]:B[name] + w].bitcast(fp16)

            th5 = sbm.tile([P, 256], fp16)
            nc.scalar.activation(out=th5[:], in_=c5[:].bitcast(fp16),
                                 func=AF.Tanh)

            b01 = ca[:, A["b01"]:A["b01"] + 2].bitcast(f32)
            bnsc = ca[:, A["bnsc"]:A["bnsc"] + 4].bitcast(f32)
            w2c = caf("w2", 1)
            qmask = caf("qmask", 8)

            h4 = sbm.tile([P, ELOC // 2], fp16)
            VA = sbe.tile([P, 2 * JW], fp16)   # [vals | alpha]
            VS = sbe.tile([P, 2 * JW], fp16)

            # ---- attention -> alpha in VA[:, JW:2JW] (DVE, early) ----
            SQ = sbe.tile([P, 2 * JW], fp16)
            nc.vector.tensor_tensor(out=SQ[:], in0=caf("qrep", 2 * JW),
                                    in1=ca[:, A["qrep"] + 2 * JW:A["qrep"] + 4 * JW].bitcast(fp16),
                                    op=AB.add)
            SXY = sbe.tile([P, 2 * JW], fp16)
            nc.vector.tensor_tensor(out=SXY[:], in0=caf("ce", 2 * JW),
                                    in1=ca[:, A["ce"] + 2 * JW:A["ce"] + 4 * JW].bitcast(fp16),
                                    op=AB.add)
            qa = sbe.tile([P, JW], fp16)
            qb = sbe.tile([P, JW], fp16)
            qc = sbe.tile([P, JW], fp16)
            for qt, cx, cy, cc in ((qa, a00, a10, v0), (qb, a01, a11, v1),
                                   (qc, u0, u1, w0s)):
                nc.vector.tensor_scalar(out=qt[:], in0=SQ[:, 0:JW], scalar1=cx,
                                        scalar2=cc, op0=AB.mult, op1=AB.add)
                nc.vector.scalar_tensor_tensor(out=qt[:], in0=SQ[:, JW:2 * JW],
                                               scalar=cy, in1=qt[:],
                                               op0=AB.mult, op1=AB.add)
            m1 = sbe.tile([P, JW], fp16)
            nc.vector.tensor_tensor(out=m1[:], in0=SXY[:, 0:JW], in1=qa[:],
                                    op=AB.mult)
            m2 = sbe.tile([P, JW], fp16)
            nc.vector.tensor_tensor(out=m2[:], in0=SXY[:, JW:2 * JW], in1=qb[:],
                                    op=AB.mult)
            nc.vector.tensor_tensor(out=m1[:], in0=m1[:], in1=m2[:], op=AB.add)
            nc.vector.tensor_tensor(out=m1[:], in0=m1[:], in1=qc[:], op=AB.add)
            nc.vector.tensor_scalar(out=m2[:], in0=m1[:], scalar1=-1.0,
                                    scalar2=None, op0=AB.mult)
            nc.vector.tensor_tensor(out=VA[:, JW:2 * JW], in0=m1[:], in1=m2[:],
                                    op=AB.max)

            dupm = cbf("dupm", nshift * JW)

            def dup_combine(col0, dtag):
                """VS[:, col0:col0+JW] = dup-group sums of VA[:, col0:col0+JW]."""
                if dmax == 0:
                    nc.vector.tensor_copy(out=VS[:, col0:col0 + JW],
                                          in_=VA[:, col0:col0 + JW])
                    return
                dsum = sbe.tile([P, nshift * JW], fp16, name=f"ds{dtag}",
                                tag=f"ds{dtag}")
                shp = psh.tile([P, nshift * JW], f32, tag="shpa", name=f"shp{dtag}")
                for s in range(nshift):
                    S = cb[:, B["shf"] + s * P:
                           B["shf"] + (s + 1) * P].bitcast(fp16)
                    nc.tensor.matmul(out=shp[:, JW * s:JW * (s + 1)],
                                     lhsT=S, rhs=VA[:, col0:col0 + JW],
                                     start=True, stop=True)
                nc.vector.tensor_tensor(out=dsum[:], in0=shp[:],
                                        in1=dupm[:], op=AB.mult)
                w = nshift * JW
                while w > JW:
                    nc.vector.tensor_tensor(out=dsum[:, 0:w // 2],
                                            in0=dsum[:, 0:w // 2],
                                            in1=dsum[:, w // 2:w], op=AB.add)
                    w //= 2
                nc.vector.tensor_tensor(out=VS[:, col0:col0 + JW],
                                        in0=VA[:, col0:col0 + JW],
                                        in1=dsum[:, 0:JW], op=AB.add)

            geo = sbe.tile([P, JW], fp16)

            def extract(h):
                """geo[:, 32h:32h+32] one-hot select from gather window h."""
                gm = sbe.tile([P, 16 * JW], fp16, name=f"gm{h}", tag=f"gm{h}")
                nc.vector.tensor_tensor(
                    out=gm[:], in0=gblk[:, 1024 * h:1024 * (h + 1)],
                    in1=cbf("mask", 32 * JW)[:, 1024 * h:1024 * (h + 1)],
                    op=AB.mult)
                with nc.allow_low_precision(reason="one-hot max-select"):
                    nc.vector.tensor_reduce(
                        out=geo[:, 32 * h:32 * (h + 1)].rearrange(
                            "p (j o) -> p j o", o=1),
                        in_=gm[:].rearrange("p (j v) -> p j v", v=32),
                        axis=mybir.AxisListType.X, op=AB.max)

            # ---- L1 (5->64 folded, 32 edge-blocks stacked on partitions) +
            # tanh + BN*tanh, feature-major; psum group g covers edges
            # [2048g, 2048(g+1)) == j in [16g, 16(g+1)) ----
            h2 = sbm.tile([P, ELOC // 2], fp16)
            vps = psr.tile([P, JW], f32, tag="rp0", name="vps", bufs=1)
            for g in range(4):
                pt = ps1.tile([P, 1024], f32, tag="l1", bufs=2)
                base = (0, 32, 64, 64)[g]
                kk = 32 if g < 2 else 64
                coff = 8 * H if g == 3 else 0
                for b in range(8):
                    nc.tensor.matmul(
                        out=pt[64 * (b % 2):64 * (b % 2) + 64,
                               256 * (b // 2):256 * (b // 2) + 256],
                        lhsT=wv[base:base + kk,
                                coff + H * b:coff + H * (b + 1)].bitcast(fp16),
                        rhs=th5[base:base + kk, :],
                        start=True, stop=True)
                nc.scalar.activation(out=h2[:, 1024 * g:1024 * (g + 1)],
                                     in_=pt[:], func=AF.Tanh,
                                     bias=b01[:, 0:1])
                nc.scalar.activation(out=h4[:, 1024 * g:1024 * (g + 1)],
                                     in_=h2[:, 1024 * g:1024 * (g + 1)],
                                     func=AF.Tanh, scale=bnsc[:, 0:1],
                                     bias=bnsc[:, 1:2])

            def tail_group(g):
                """L2 vals + relu for j in [16g, 16g+16)."""
                for j in range(16 * g, 16 * (g + 1)):
                    t = (j // 2) % 2
                    cols = 1024 * g + 256 * ((j % 16) // 4) + 128 * (j % 2)
                    nc.tensor.matmul(out=vps[:, j:j + 1],
                                     lhsT=h4[64 * t:64 * t + 64,
                                             cols:cols + 128],
                                     rhs=w2c[64 * t:64 * t + 64, :],
                                     start=True, stop=True)
                js = slice(16 * g, 16 * (g + 1))
                nc.vector.tensor_scalar(out=VA[:, js], in0=vps[:, js],
                                        scalar1=b2f, scalar2=0.0,
                                        op0=AB.add, op1=AB.max)

            def dup_half(hh):
                """VS[:, 32hh:32hh+32] = dup-combined vals for j-half hh."""
                js = slice(32 * hh, 32 * (hh + 1))
                if dmax == 0:
                    nc.vector.tensor_copy(out=VS[:, js], in_=VA[:, js])
                    return
                dsh = sbe.tile([P, nshift * 32], fp16, name=f"dsh{hh}",
                               tag=f"dsh{hh}")
                shh = psh.tile([P, nshift * 32], f32, tag="shpa",
                               name=f"shh{hh}")
                for s in range(nshift):
                    S = cb[:, B["shf"] + s * P:
                           B["shf"] + (s + 1) * P].bitcast(fp16)
                    nc.tensor.matmul(out=shh[:, 32 * s:32 * (s + 1)],
                                     lhsT=S, rhs=VA[:, js],
                                     start=True, stop=True)
                nc.vector.tensor_tensor(
                    out=dsh[:].rearrange("p (s j) -> p s j", j=32),
                    in0=shh[:].rearrange("p (s j) -> p s j", j=32),
                    in1=dupm[:].rearrange("p (s j) -> p s j", j=JW)[:, :, js],
                    op=AB.mult)
                w = nshift * 32
                while w > 32:
                    nc.vector.tensor_tensor(out=dsh[:, 0:w // 2],
                                            in0=dsh[:, 0:w // 2],
                                            in1=dsh[:, w // 2:w], op=AB.add)
                    w //= 2
                nc.vector.tensor_tensor(out=VS[:, js], in0=VA[:, js],
                                        in1=dsh[:, 0:32], op=AB.add)

            # alpha-half dup combining; then interleave geodesic extraction
            # with the per-group vals tails in data-arrival order
            dup_combine(JW, "a")
            extract(0)
            tail_group(0)
            extract(1)
            wdec = sbe.tile([P, JW], fp16)
            nc.vector.tensor_tensor(out=wdec[:], in0=VS[:, JW:2 * JW],
                                    in1=geo[:], op=AB.mult)
            # fold the dup/angle rep factor into the exp argument
            # (dec = exp(-wdec)*rep == exp(-(wdec - ln rep)))
            wdecm = sbe.tile([P, JW], fp16)
            nc.vector.tensor_tensor(out=wdecm[:], in0=wdec[:],
                                    in1=cbf("rep", JW), op=AB.add)
            tail_group(1)
            tail_group(2)
            # gate exp on the last h4 quarter (0*h4) so the in-order Act
            # stream never stalls mid-MLP waiting for the geodesic chain
            dec = sbe.tile([P, JW], fp16)
            nc.scalar.activation(out=dec[:], in_=wdecm[:], func=AF.Exp,
                                 scale=-1.0)
            dup_half(0)
            tail_group(3)
            # dec8[p, 8j+q] = dec[p, j] * qmask[p, q], split in j-halves
            dec8 = sbe.tile([P, 8 * JW], fp16)

            def dec8_half(hh, eng):
                js = slice(JW // 2 * hh, JW // 2 * (hh + 1))
                eng.tensor_tensor(
                    out=dec8[:].rearrange("p (j q) -> p j q", q=8)[:, js],
                    in0=dec[:].rearrange("p (j o) -> p j o", o=1)
                        .to_broadcast([P, JW, 8])[:, js],
                    in1=qmask[:, None, :].to_broadcast([P, JW, 8])[:, js],
                    op=AB.mult)

            dec8_half(1, nc.gpsimd)
            dec8_half(0, nc.vector)

            # ---- wds (planar [p, 512v + 8j + q], built per j-half); squaring
            # is exact because the one-hot replication mask is idempotent ----
            wds = sbe.tile([P, 3 * 8 * JW], fp16)
            HJ = JW // 2

            def wds_half(hh):
                js = slice(HJ * hh, HJ * (hh + 1))
                cs = slice(8 * HJ * hh, 8 * HJ * (hh + 1))
                nc.vector.tensor_tensor(
                    out=wds[:, 0:8 * JW].rearrange("p (j q) -> p j q", q=8)[:, js],
                    in0=VS[:, 0:JW].rearrange("p (j o) -> p j o", o=1)
                        .to_broadcast([P, JW, 8])[:, js],
                    in1=dec8[:].rearrange("p (j q) -> p j q", q=8)[:, js],
                    op=AB.mult)
                nc.vector.tensor_tensor(
                    out=wds[:, 8 * JW:16 * JW][:, cs],
                    in0=wds[:, 0:8 * JW][:, cs],
                    in1=wds[:, 0:8 * JW][:, cs], op=AB.mult)
                nc.vector.tensor_tensor(
                    out=wds[:, 16 * JW:24 * JW][:, cs],
                    in0=wds[:, 0:8 * JW][:, cs],
                    in1=wds[:, 8 * JW:16 * JW][:, cs], op=AB.mult)

            dup_half(1)
            wds_half(0)
            wds_half(1)

            # ---- reduce: per (j, view) matmuls into per-view psum banks ----
            rpv = [psr.tile([F, 8 * JW], f32, tag=f"rp{v}", name=f"rp{v}")
                   for v in range(n_views)]
            for j in range(JW):
                for v in range(n_views):
                    nc.tensor.matmul(
                        out=rpv[v][:, 8 * j:8 * (j + 1)],
                        lhsT=xg[:, F * j:F * (j + 1)],
                        rhs=wds[:, 8 * JW * v + 8 * j:8 * JW * v + 8 * (j + 1)],
                        start=True, stop=True)

            # ---- per-(view, half) copies split across DVE/Act + one store ----
            outT = sbo.tile([n_views * F, NLOC], fp16)
            # Act is idle after the MLP, DVE is still finishing wds: Act
            # takes 4 of the 6 half-copies
            for hh in range(2):
                rs = slice(NLOC // 2 * hh, NLOC // 2 * (hh + 1))
                for v in range(n_views):
                    if (v + hh) % 2 == 0:
                        nc.scalar.copy(out=outT[F * v:F * (v + 1), rs],
                                       in_=rpv[v][:, rs])
                    else:
                        nc.vector.tensor_copy(out=outT[F * v:F * (v + 1), rs],
                                              in_=rpv[v][:, rs])
            nc.sync.dma_start(out=out[:, 0:NLOC // 2],
                              in_=outT[:, 0:NLOC // 2])
            nc.sync.dma_start(out=out[:, NLOC // 2:NLOC],
                              in_=outT[:, NLOC // 2:NLOC])

    nc.compile()
    return nc


def _prepare(inputs):
    """Host-side staging: edge sort, gather tables/indices, masks, weight folds."""
    x = np.asarray(inputs["x"], np.float32)
    coor = np.asarray(inputs["local_graph_coor"], np.float32)
    sparse_idx = np.asarray(inputs["sparse_idx"])
    geodesic = np.asarray(inputs["geodesic"], np.float32)
    angle_ratio = float(np.asarray(inputs["angle_ratio"]).ravel()[0])
    Wq = np.asarray(inputs["Wq"], np.float32); bq = np.asarray(inputs["bq"], np.float32)
    Wk = np.asarray(inputs["Wk"], np.float32); bk = np.asarray(inputs["bk"], np.float32)
    W0 = np.asarray(inputs["W0"], np.float32); b0 = np.asarray(inputs["b0"], np.float32)
    W1 = np.asarray(inputs["W1"], np.float32); b1 = np.asarray(inputs["b1"], np.float32)
    bn_g = np.asarray(inputs["bn_g"], np.float32); bn_b = np.asarray(inputs["bn_b"], np.float32)
    bn_m = np.asarray(inputs["bn_m"], np.float32); bn_v = np.asarray(inputs["bn_v"], np.float32)
    W2 = np.asarray(inputs["W2"], np.float32); b2 = np.asarray(inputs["b2"], np.float32)
    n_views = int(np.asarray(inputs["max_view"]).ravel()[0])

    col = np.asarray(sparse_idx[1], np.int64).reshape(N, KN)
    order = np.argsort(col, axis=1, kind="stable")
    col_s = np.take_along_axis(col, order, axis=1)                  # (N, K)
    eidx = (np.arange(N)[:, None] * KN + order).reshape(-1)
    coor_s = coor[eidx]                                             # (E, 4)

    same_prev = np.zeros((N, KN), bool)
    same_prev[:, 1:] = col_s[:, 1:] == col_s[:, :-1]
    rep = (~same_prev).astype(np.float32) * angle_ratio
    run = np.zeros((N, KN), np.int64)
    for k in range(1, KN):
        run[:, k] = np.where(same_prev[:, k], run[:, k - 1] + 1, 0)
    dmax = int(run.max())

    Aq = Wq @ Wk.T
    u = Wq @ bk
    vv = Wk @ bq
    attn_consts = (Aq[0, 0], Aq[0, 1], Aq[1, 0], Aq[1, 1], u[0], u[1],
                   vv[0], vv[1], float(bq @ bk), float(b2.ravel()[0]))

    bns = (bn_g / np.sqrt(bn_v + BN_EPS)).astype(np.float32)
    bnc = (bn_b - bn_m * bns).astype(np.float32)
    W01 = (W0 @ W1).astype(np.float32)                              # (4, H)
    b01 = (b0 @ W1 + b1).astype(np.float32)                         # (H,)

    A, G, B = _layout(dmax)
    pidx = np.arange(P)

    def wrap16(lst):
        lst = np.asarray(lst, np.int64)
        return np.tile(lst.reshape(len(lst) // 16, 16).T, (8, 1)).astype(np.int16)

    xtab = np.zeros((N, 128), np.float16)
    xtab[:, :F] = x.astype(np.float16)

    in_maps = []
    for c in range(NCORES):
        r0 = c * NLOC
        colc = col_s[r0:r0 + NLOC].reshape(-1)                      # (ELOC,) l-order
        coorc = coor_s[r0 * KN:(r0 + NLOC) * KN]                    # (ELOC, 4)
        repc = rep[r0:r0 + NLOC].reshape(-1)
        gsh = geodesic[r0:r0 + NLOC]                                # (512, 4096)
        rows_l = np.arange(ELOC) // KN                              # local row per edge
        k_l = np.arange(ELOC) % KN

        ce5_a = np.zeros((P, 256), np.float16)
        c5v = coorc.astype(np.float16).reshape(32, 256, 4)   # (q, col, c)
        for q in range(32):
            p0 = 32 * (q // 8) + 4 * (q % 8)
            ce5_a[p0:p0 + 4, :] = c5v[q].T
        cst0_a = np.zeros((P, A["_W"]), np.float16)
        ce = coorc.astype(np.float16)                               # (ELOC, 4)
        cst0_a[:, A["ce"]:A["ce"] + 4 * JW] = \
            ce.reshape(JW, P, 4).transpose(1, 2, 0).reshape(P, 4 * JW)
        qco = coor[(np.arange(r0, r0 + NLOC)) * KN].astype(np.float16)  # slot0 coor
        qrep = qco[rows_l]                                          # (ELOC, 4)
        cst0_a[:, A["qrep"]:A["qrep"] + 4 * JW] = \
            qrep.reshape(JW, P, 4).transpose(1, 2, 0).reshape(P, 4 * JW)
        w01v_a = np.zeros((P, 16 * H), np.float16)
        W01h = W01.astype(np.float16)
        for b in range(8):
            for r0_ in (0, 32, 64):                 # groups 0, 1, 2
                w01v_a[r0_ + 4 * b:r0_ + 4 * b + 4, H * b:H * (b + 1)] = W01h
            w01v_a[96 + 4 * b:96 + 4 * b + 4,       # group 3 (base-64 upper)
                   8 * H + H * b:8 * H + H * (b + 1)] = W01h
        cst0_a[:, A["qmask"]:A["qmask"] + 8] = \
            (pidx[:, None] // 16 == np.arange(8)[None, :]).astype(np.float16)
        cst0_a[:, A["w2"]:A["w2"] + 1] = \
            np.tile(W2[:, 0], 2).astype(np.float16)[:, None]
        bnsc = np.stack([np.tile(bns, 2), np.tile(bnc, 2)], 1).astype(np.float32)
        cst0_a[:, A["bnsc"]:A["bnsc"] + 4] = bnsc.view(np.float16)
        cst0_a[:, A["b01"]:A["b01"] + 2] = \
            np.tile(b01, 2).astype(np.float32)[:, None].view(np.float16)

        cstg_a = np.zeros((P, G["_W"]), np.float16)
        cstg_a[:, G["gx"]:G["gx"] + ELOC // 16] = wrap16(colc).view(np.float16)
        for h in range(2):
            el = np.arange(h * ELOC // 2, (h + 1) * ELOC // 2)
            gidx = (rows_l[el] % 256) * 128 + colc[el] // 32
            cstg_a[:, G["gg" + str(h)]:G["gg" + str(h)] + ELOC // 32] = \
                wrap16(gidx).view(np.float16)

        cstb_a = np.zeros((P, B["_W"]), np.float16)
        onehot = (colc % 32)[:, None] == np.arange(32)[None, :]     # (ELOC, 32)
        cstb_a[:, B["mask"]:B["mask"] + 32 * JW] = \
            onehot.astype(np.float16).reshape(JW, P, 32).transpose(1, 0, 2) \
            .reshape(P, 32 * JW)
        for s in range(2 * dmax):
            d = s + 1 if s < dmax else -(s - dmax + 1)
            # shift matrix: out[m, c] = VA[m+d, c]  -> S[k, m] = [k == m+d]
            S = (pidx[:, None] == pidx[None, :] + d).astype(np.float16)
            cstb_a[:, B["shf"] + s * P:B["shf"] + (s + 1) * P] = S
            # mask for dst edge l=128j+p absorbing l+d (same row & col)
            msk = np.zeros(ELOC, np.float32)
            if d > 0:
                msk[:ELOC - d] = ((k_l[:ELOC - d] + d < KN) &
                                  (colc[d:] == colc[:-d]))
            else:
                dd = -d
                msk[dd:] = ((k_l[dd:] - dd >= 0) & (colc[:-dd] == colc[dd:]))
            cstb_a[:, B["dupm"] + s * JW:B["dupm"] + (s + 1) * JW] = \
                np.ascontiguousarray(msk.reshape(JW, P).T).astype(np.float16)
        lnrepn = np.where(repc > 0, -np.log(np.maximum(repc, 1e-30)),
                          60000.0).astype(np.float32)
        cstb_a[:, B["rep"]:B["rep"] + JW] = \
            np.ascontiguousarray(lnrepn.reshape(JW, P).T).astype(np.float16)

        gt = []
        for h in range(2):
            g4 = gsh[256 * h:256 * (h + 1)].reshape(NLOC * N // 64, 32)
            gta = np.zeros((NLOC * N // 64, 128), np.float16)
            gta[:, :32] = g4.astype(np.float16)
            gt.append(gta)

        in_maps.append({"ce5t": ce5_a.view(np.int16),
                        "w01t": w01v_a.view(np.int16),
                        "cst0": cst0_a.view(np.int16),
                        "cstg": cstg_a.view(np.int16),
                        "cstb": cstb_a.view(np.int16),
                        "xtab": xtab, "gtab0": gt[0], "gtab1": gt[1]})
    return in_maps, dmax, n_views, attn_consts


def kernel(**inputs):
    from concourse.bass_utils import run_bass_kernel_spmd
    in_maps, dmax, n_views, attn_consts = _prepare(inputs)
    nc = _build_program(dmax, n_views, attn_consts, sim=False)
    res = run_bass_kernel_spmd(nc, in_maps, list(range(NCORES)))
    return np.vstack([np.asarray(res.results[c]["out"]).T.astype(np.float32)
                      for c in range(NCORES)])
